# revision 5
# baseline (speedup 1.0000x reference)
"""Trainium2 Bass kernel for a single causal attention head.

  q = x @ Wq.T; k = pos_emb @ Wk.T; v = x @ Wv.T
  out = softmax(causal(q @ k.T / sqrt(E))) @ v

Sharding (8 cores): core c -> (batch b = c//2, half h = c%2). Core h owns the
interleaved 128-row blocks {2j+h} of batch b (queries AND keys) so causal work
is balanced across the pair. Each core projects Q/K/V for its own rows;
attention over the core's OWN key half reads K/V straight from SBUF (no
collective), while the peer half is recovered rank-uniformly as
AllReduce_pair(add) - own on the vector engine. Masks encode the own/other
causal boundary as data, so the program has no rank-dependent control flow.

Precision: Q and K projections run in fp8(e4m3) DoubleRow mode (two E-chunks
packed per matmul, 0.5 cycles per output column); scores, V and attn@v stay
fp16 (fp32 PSUM accumulate) — validated ~1.7e-2 max rel err vs fp64 on the
reference inputs. Softmax denominators come from quad-summed exp tiles (DVE)
feeding one ones-vector matmul per quad instead of one per tile.

All activations are fed host-transposed and partition-major ([P, EC, T_core])
so every matmul has the contraction dim on partitions with zero on-chip
activation transposes, and each stream DMA moves contiguous multi-KB rows.
"""

import os
import sys
from contextlib import ExitStack
from dataclasses import dataclass

import numpy as np


def _ensure_path():
    try:
        import concourse.bass  # noqa: F401
    except ImportError:
        for p in ("/opt/trn_rl_repo", "/root/.axon_site/_ro/trn_rl_repo"):
            if os.path.isdir(p) and p not in sys.path:
                sys.path.insert(0, p)


_ensure_path()

import concourse.bass as bass  # noqa: E402
import concourse.mybir as mybir  # noqa: E402
import concourse.tile as tile  # noqa: E402
from concourse.masks import make_identity  # noqa: E402

P = 128
F8 = mybir.dt.float8e4
F16 = mybir.dt.float16
F32 = mybir.dt.float32
DR = mybir.MatmulPerfMode.DoubleRow


@dataclass(frozen=True)
class Cfg:
    B: int = 4
    T: int = 2048
    E: int = 4096
    H: int = 128
    QGB: int = 4  # 128-blocks per query group (matmul free dim = QGB*P)

    @property
    def NB(self):  # key/query 128-blocks per core per half
        return self.T // (2 * P)

    @property
    def TB(self):  # rows per core
        return self.NB * P

    @property
    def NQG(self):  # query groups per core
        return self.NB // self.QGB

    @property
    def QG(self):  # queries per group
        return self.QGB * P

    @property
    def EC(self):  # contraction chunks
        return self.E // P

    @property
    def NPAIR(self):  # fp8 DoubleRow chunk pairs
        return self.EC // 2


FULL = Cfg()

# walrus CoreV3 setupSyncWait rejects instructions carrying more than
# MAX_SYNC_WAITS wait conditions; excess waits are hoisted onto injected
# same-engine NoOps placed immediately before the offender.
MAX_SYNC_WAITS = 1


def _dedupe_ldweights(nc: bass.Bass):
    """Drop PE Ldweights whose stationary operand is identical to the weights
    already loaded. Any sync conditions move onto the following PE
    instruction."""
    import orjson

    n = 0
    for fn in nc.m.functions:
        for bb in fn.blocks:
            out = []
            last_sig = None
            pending_sync = None
            for inst in bb.instructions:
                if getattr(inst, "engine", None) != mybir.EngineType.PE:
                    out.append(inst)
                    continue
                d = orjson.loads(nc.instruction_to_json(inst))
                if d["opcode"] == "Ldweights":
                    sig = orjson.dumps(
                        [d.get("ins"), d.get("tile_position"), d.get("tile_size")]
                    )
                    if sig == last_sig:
                        si = inst.sync_info
                        if si and (si.on_wait or si.on_update):
                            pending_sync = si
                        n += 1
                        continue  # drop
                    last_sig = sig
                if pending_sync is not None:
                    si = inst.sync_info
                    if si is None:
                        inst.sync_info = pending_sync
                    else:
                        si.on_wait = list(pending_sync.on_wait) + list(si.on_wait)
                        si.on_update = list(pending_sync.on_update) + list(
                            si.on_update
                        )
                    pending_sync = None
                out.append(inst)
            assert pending_sync is None
            bb.instructions[:] = out
    return n


def _split_sync_waits(nc: bass.Bass, maxw: int = MAX_SYNC_WAITS):
    n = 0
    for fn in nc.m.functions:
        for bb in fn.blocks:
            out = []
            for inst in bb.instructions:
                si = inst.sync_info
                waits = list(si.on_wait) if si and si.on_wait else []
                if len(waits) > maxw:
                    excess, keep = waits[:-maxw], waits[-maxw:]
                    for k in range(0, len(excess), maxw):
                        carrier = mybir.InstNoOp(
                            name=f"{inst.name}-wsplit{n}",
                            engine=inst.engine,
                            ins=[],
                            outs=[],
                            sync_info=mybir.SyncInfo(
                                on_wait=excess[k : k + maxw], on_update=[]
                            ),
                        )
                        n += 1
                        out.append(carrier)
                    si.on_wait = keep
                out.append(inst)
            bb.instructions[:] = out
    return n


def build(cfg: Cfg, mock_cc: bool = False) -> bass.Bass:
    assert cfg.H == P
    TB, NB, EC, QG, QGB, NQG, H, NPAIR = (
        cfg.TB, cfg.NB, cfg.EC, cfg.QG, cfg.QGB, cfg.NQG, cfg.H, cfg.NPAIR,
    )
    KV = TB * H  # fp16 elements of one of {kT, v} local halves

    nc = bass.Bass("TRN2", target_bir_lowering=False, debug=False, num_devices=8)

    # p-major streams: [p, c, t] so one DMA row per partition covers a whole
    # super-chunk contiguously (multi-KB descriptors)
    pe8 = nc.dram_tensor("pe8", [P, EC, TB], F8, kind="ExternalInput").ap()
    x8 = nc.dram_tensor("x8", [P, EC, TB], F8, kind="ExternalInput").ap()
    xP = nc.dram_tensor("xP", [P, EC, TB], F16, kind="ExternalInput").ap()
    # per-pair interleave [wk8_m | wq8_m] with [2(j), H] inner layout
    w8 = nc.dram_tensor("w8", [P, NPAIR * 2 * 2 * H], F8, kind="ExternalInput").ap()
    wv = nc.dram_tensor("wv", [P, EC * H], F16, kind="ExternalInput").ap()
    qmask = nc.dram_tensor("qmask", [P, 2 * P], F16, kind="ExternalInput").ap()
    outT = nc.dram_tensor("outT", [H, TB], F16, kind="ExternalOutput").ap()

    cc_k_in = nc.dram_tensor("cc_k_in", [KV], F16).ap()
    cc_k_out = nc.dram_tensor("cc_k_out", [KV], F16).ap()
    cc_v_in = nc.dram_tensor("cc_v_in", [KV], F16).ap()
    cc_v_out = nc.dram_tensor("cc_v_out", [KV], F16).ap()

    scale = 1.0 / np.sqrt(float(cfg.E))
    groups = [[0, 1], [2, 3], [4, 5], [6, 7]]

    # super-chunks in PAIRS of 128-chunks (DoubleRow consumes 2 at a time);
    # small first chunks so the PE starts almost immediately
    PSCS = []
    rem = NPAIR
    for want in [1, 1] + [2] * NPAIR:
        if rem == 0:
            break
        s = min(want, rem)
        PSCS.append(s)
        rem -= s
    NSC = len(PSCS)
    POFF = [sum(PSCS[:i]) for i in range(NSC)]

    NT = 2
    FD = 512  # projection matmul free dim (psum tile)

    with tile.TileContext(nc) as tc, ExitStack() as ctx:
        consts = ctx.enter_context(tc.tile_pool(name="consts", bufs=1))
        big = ctx.enter_context(tc.tile_pool(name="big", bufs=1))
        pe8_pool = ctx.enter_context(tc.tile_pool(name="pe8p", bufs=3))
        x8_pool = ctx.enter_context(tc.tile_pool(name="x8p", bufs=5))
        xp_pool = ctx.enter_context(tc.tile_pool(name="xpp", bufs=5))
        e_pool = ctx.enter_context(tc.tile_pool(name="eT", bufs=4 * QGB * NQG + 10))
        sm = ctx.enter_context(tc.tile_pool(name="sm", bufs=2))

        proj_ctx = ExitStack()
        pp = proj_ctx.enter_context(tc.tile_pool(name="pp", bufs=6, space="PSUM"))
        tr_ps_pool = proj_ctx.enter_context(
            tc.tile_pool(name="trp", bufs=2, space="PSUM")
        )

        # ---- constants ----
        ones_col = consts.tile([P, 1], F16, tag="ones_col")
        nc.any.memset(ones_col[:], 1.0)
        ones_row = consts.tile([1, P], F32, tag="ones_row")
        nc.any.memset(ones_row[:], 1.0)
        ident = consts.tile([P, P], F16, tag="ident")
        make_identity(nc, ident[:])
        # preload the ACT Exp table during the DMA belt so the first attention
        # exp doesn't pay the cold-table load on the critical path
        warm = consts.tile([P, 1], F16, tag="warm")
        nc.scalar.activation(
            warm[:], ones_col[:], mybir.ActivationFunctionType.Exp
        )

        w8_sb = consts.tile([P, NPAIR, 2, 2, H], F8, tag="w8")
        wv_sb = consts.tile([P, EC, H], F16, tag="wv")
        qm_sb = consts.tile([P, 2 * P], F16, tag="qm")

        # ---- belt DMA queue plan ----
        # scalar (ACT) queue: pe8 stream, then x8 stream, then qmask.
        # sync (SP) queue: all w8 up front, then xP stream with wv slices
        # riding one super-chunk ahead; outT rides the tail.
        # gpsimd (SWDGE) queue: collective bounces + readbacks only.
        w8v = w8.rearrange("p (m s j h) -> p m s j h", s=2, j=2, h=H)
        wvv = wv.rearrange("p (e h) -> p e h", h=H)

        pe8_tiles = {}
        x8_tiles = {}
        xp_tiles = {}
        for sc in range(NSC):
            n = PSCS[sc]
            off = POFF[sc]
            t = pe8_pool.tile([P, 2 * n, TB], F8, tag="pe8", name=f"pe8_{sc}")
            nc.scalar.dma_start(t[:], pe8[:, 2 * off : 2 * (off + n), :])
            pe8_tiles[sc] = t
        for sc in range(NSC):
            n = PSCS[sc]
            off = POFF[sc]
            t = x8_pool.tile([P, 2 * n, TB], F8, tag="x8", name=f"x8_{sc}")
            nc.scalar.dma_start(t[:], x8[:, 2 * off : 2 * (off + n), :])
            x8_tiles[sc] = t
        nc.scalar.dma_start(qm_sb[:], qmask)

        nc.sync.dma_start(w8_sb[:], w8v)
        for sc in range(NSC):
            n = PSCS[sc]
            off = POFF[sc]
            nc.sync.dma_start(
                wv_sb[:, 2 * off : 2 * (off + n), :],
                wvv[:, 2 * off : 2 * (off + n), :],
            )
            t = xp_pool.tile([P, 2 * n, TB], F16, tag="xp", name=f"xp_{sc}")
            nc.sync.dma_start(t[:], xP[:, 2 * off : 2 * (off + n), :])
            xp_tiles[sc] = t

        k_ps = [pp.tile([P, FD], F32, tag="pp", name=f"k_ps{i}") for i in range(NT)]
        v_ps = [pp.tile([P, FD], F32, tag="pp", name=f"v_ps{i}") for i in range(NT)]
        q_ps = [pp.tile([P, FD], F32, tag="pp", name=f"q_ps{i}") for i in range(NT)]

        def proj8(ps, s, t, sc):
            # fp8 DoubleRow: stationary [P, 2, H] (pair), moving [P, 2, FD]
            for i in range(PSCS[sc]):
                m = POFF[sc] + i
                for it in range(NT):
                    nc.tensor.matmul(
                        ps[it][:],
                        w8_sb[:, m, s],
                        t[:, 2 * i : 2 * i + 2, it * FD : (it + 1) * FD],
                        start=(m == 0),
                        stop=(m == NPAIR - 1),
                        perf_mode=DR,
                    )

        def projv(sc):
            for i in range(2 * PSCS[sc]):
                e = 2 * POFF[sc] + i
                for it in range(NT):
                    nc.tensor.matmul(
                        v_ps[it][:],
                        wv_sb[:, e, :],
                        xp_tiles[sc][:, i, it * FD : (it + 1) * FD],
                        start=(e == 0),
                        stop=(e == EC - 1),
                    )

        # ---- phase 1: projections. K at pe8-stream pace (fastest queue),
        # V interleaved at xP pace, Q rides the x8 tail.
        for sc in range(NSC):
            proj8(k_ps, 0, pe8_tiles[sc], sc)
            if sc >= 2:
                projv(sc - 2)
        # K done -> own half to SBUF, peer exchange in the background
        with tc.high_priority():
            kT_own = big.tile([P, TB], F16, tag="kT_own")
            for i in range(NT):
                nc.vector.tensor_copy(kT_own[:, i * FD : (i + 1) * FD], k_ps[i][:])
            nc.gpsimd.dma_start(
                cc_k_in.rearrange("(h t) -> h t", t=TB), kT_own[:]
            )
            if mock_cc:
                nc.gpsimd.dma_start(cc_k_out[:], cc_k_in[:])
            else:
                nc.gpsimd.collective_compute(
                    "AllReduce",
                    mybir.AluOpType.add,
                    replica_groups=groups,
                    ins=[cc_k_in[:]],
                    outs=[cc_k_out[:]],
                )
            kT_sum = big.tile([P, TB], F16, tag="kT_sum")
            nc.gpsimd.dma_start(
                kT_sum[:], cc_k_out.rearrange("(h t) -> h t", t=TB)
            )
            kT_oth = big.tile([P, TB], F16, tag="kT_oth")
            nc.vector.tensor_sub(kT_oth[:], kT_sum[:], kT_own[:])

        for sc in range(NSC):
            proj8(q_ps, 1, x8_tiles[sc], sc)
            if NSC - 2 + sc < NSC:
                projv(NSC - 2 + sc)

        qT_sb = big.tile([P, TB], F16, tag="qT")
        for i in range(NT):
            nc.vector.tensor_copy(qT_sb[:, i * FD : (i + 1) * FD], q_ps[i][:])

        # ---- V done: transpose to natural layout, exchange ----
        vT_loc = big.tile([P, TB], F16, tag="vT_loc")
        for i in range(NT):
            nc.vector.tensor_copy(vT_loc[:, i * FD : (i + 1) * FD], v_ps[i][:])
        v_own = big.tile([P, NB, H], F16, tag="v_own")
        for c in range(NB):
            t_ps = tr_ps_pool.tile([P, P], F16, tag="tr")
            nc.tensor.transpose(t_ps[:], vT_loc[:, c * P : (c + 1) * P], ident[:])
            nc.vector.tensor_copy(v_own[:, c, :], t_ps[:])
        nc.gpsimd.dma_start(
            cc_v_in.rearrange("(p c h) -> p c h", p=P, h=H), v_own[:]
        )
        if mock_cc:
            nc.gpsimd.dma_start(cc_v_out[:], cc_v_in[:])
        else:
            nc.gpsimd.collective_compute(
                "AllReduce",
                mybir.AluOpType.add,
                replica_groups=groups,
                ins=[cc_v_in[:]],
                outs=[cc_v_out[:]],
            )
        v_sum = big.tile([P, NB, H], F16, tag="v_sum")
        nc.gpsimd.dma_start(
            v_sum[:], cc_v_out.rearrange("(p c h) -> p c h", p=P, h=H)
        )
        v_oth = big.tile([P, NB, H], F16, tag="v_oth")
        nc.vector.tensor_sub(v_oth[:], v_sum[:], v_own[:])

        # ---- phase 2: attention ----
        # halves: r=0 own keys (SBUF-resident), r=1 peer keys (exchanged).
        # Key order within softmax is irrelevant; qmask cols [:P] mask the
        # own-half diagonal tiles (tril), cols [P:] the peer-half diagonal
        # (zeros for h=0, ones for h=1).
        proj_ctx.close()  # release projection PSUM banks
        sT_pool = ctx.enter_context(tc.tile_pool(name="sTp", bufs=4, space="PSUM"))
        o_pool = ctx.enter_context(tc.tile_pool(name="op", bufs=2, space="PSUM"))
        d_pool = ctx.enter_context(tc.tile_pool(name="dp", bufs=2, space="PSUM"))

        o_ps = {}
        d_ps = {}
        e_tiles = {g: [] for g in range(NQG)}  # (kslot, eT, col0)
        esums = {g: [] for g in range(NQG)}  # quad-summed exp tiles
        nk = {g: QGB * (g + 1) for g in range(NQG)}
        for g in range(NQG):
            o_ps[g] = o_pool.tile([P, QG], F32, tag="o", name=f"o_ps{g}")
            d_ps[g] = d_pool.tile([1, QG], F32, tag="d", name=f"d_ps{g}")

        def mm1_half(r, kT_half):
            # kslot-outer interleave so consecutive MM1s share one Ldweights
            for c in range(NB):
                for g in range(NQG):
                    if c >= nk[g]:
                        continue
                    col0 = (c - QGB * g) * P if c >= QGB * g else 0
                    sT = sT_pool.tile(
                        [P, QG], F32, tag="sT", name=f"sT_{g}_{r}_{c}"
                    )
                    nc.tensor.matmul(
                        sT[:, col0:],
                        kT_half[:, c * P : (c + 1) * P],
                        qT_sb[:, g * QG + col0 : (g + 1) * QG],
                        start=True,
                        stop=True,
                    )
                    eT = e_pool.tile(
                        [P, QG], F16, tag="eT", name=f"eT_{g}_{r}_{c}"
                    )
                    if c >= QGB * g:
                        nc.scalar.activation(
                            eT[:, col0:], sT[:, col0:],
                            mybir.ActivationFunctionType.Exp, scale=scale,
                        )
                        nc.vector.tensor_mul(
                            eT[:, col0 : col0 + P],
                            eT[:, col0 : col0 + P],
                            qm_sb[:, r * P : (r + 1) * P],
                        )
                    else:
                        nc.scalar.activation(
                            eT[:], sT[:], mybir.ActivationFunctionType.Exp,
                            scale=scale,
                        )
                    e_tiles[g].append((r * NB + c, eT, col0))
                    # quad-summed denominators on DVE: tiles arrive with
                    # monotonically nondecreasing col0 inside each quad, so
                    # the partial sum below col0 stays exact
                    qidx = len(e_tiles[g]) - 1
                    if qidx % QGB == 0:
                        es = e_pool.tile(
                            [P, QG], F16, tag="eT", name=f"es_{g}_{r}_{c}"
                        )
                        nc.vector.tensor_copy(es[:, col0:], eT[:, col0:])
                        esums[g].append((es, col0))
                    else:
                        es, _ = esums[g][-1]
                        nc.vector.tensor_add(
                            es[:, col0:], es[:, col0:], eT[:, col0:]
                        )

        mm1_half(0, kT_own)
        mm1_half(1, kT_oth)

        # denominators: one ones-matmul per quad
        for g in range(NQG):
            for i, (es, col0) in enumerate(esums[g]):
                nc.tensor.matmul(
                    d_ps[g][:], ones_col[:], es[:],
                    start=(i == 0), stop=(i == len(esums[g]) - 1),
                )

        # attn @ v: own tiles first (v_own ready long before v_oth),
        # kslot-outer so g=0/g=1 share each v stationary
        mm3_idx = {g: 0 for g in range(NQG)}
        n_mm3 = {g: 2 * nk[g] for g in range(NQG)}

        def mm3_half(r, v_half):
            for c in range(NB):
                for g in range(NQG):
                    if c >= nk[g]:
                        continue
                    kslot = r * NB + c
                    eT, col0 = next(
                        (e, c0) for (ks, e, c0) in e_tiles[g] if ks == kslot
                    )
                    nc.tensor.matmul(
                        o_ps[g][:, col0:], v_half[:, c, :], eT[:, col0:],
                        start=(mm3_idx[g] == 0),
                        stop=(mm3_idx[g] == n_mm3[g] - 1),
                    )
                    mm3_idx[g] += 1

        mm3_half(0, v_own)
        mm3_half(1, v_oth)

        for g in range(NQG):
            rec = sm.tile([1, QG], F32, tag="rec", name=f"rec{g}")
            nc.vector.reciprocal(rec[:], d_ps[g][:])
            bc_ps = sT_pool.tile([P, QG], F32, tag="sT", name=f"bc_ps{g}")
            nc.tensor.matmul(bc_ps[:], ones_row[:], rec[:], start=True, stop=True)
            bc_sb = sm.tile([P, QG], F32, tag="bcs", name=f"bc_sb{g}")
            nc.vector.tensor_copy(bc_sb[:], bc_ps[:])
            oT = sm.tile([P, QG], F16, tag="oT", name=f"oT{g}")
            nc.vector.tensor_mul(oT[:], o_ps[g][:], bc_sb[:])
            nc.sync.dma_start(outT[:, g * QG : (g + 1) * QG], oT[:])

    return nc


def _core_rows(cfg: Cfg, h: int) -> np.ndarray:
    j = np.arange(cfg.TB)
    return ((j // P) * 2 + h) * P + (j % P)


def _pmajor(cfg: Cfg, a: np.ndarray, dt) -> np.ndarray:
    # [TB, E] rows -> [P, EC, TB] with [p, c, t] = a[t, c*P + p]
    return np.ascontiguousarray(
        a.T.reshape(cfg.EC, P, cfg.TB).transpose(1, 0, 2)
    ).astype(dt)


def _w8_layout(cfg: Cfg, Wk, Wq, dt) -> np.ndarray:
    # [P, NPAIR, 2(proj: k,q), 2(j), H] with [p, m, s, j, h] = W_s[h, (2m+j)P+p]
    def lay(W):  # [P, NPAIR, 2, H]
        return W.T.reshape(cfg.NPAIR, 2, P, cfg.H).transpose(2, 0, 1, 3)

    out = np.empty((P, cfg.NPAIR, 2, 2, cfg.H), np.float32)
    out[:, :, 0] = lay(Wk)
    out[:, :, 1] = lay(Wq)
    return np.ascontiguousarray(
        out.reshape(P, cfg.NPAIR * 2 * 2 * cfg.H)
    ).astype(dt)


def _wv_layout(cfg: Cfg, Wv) -> np.ndarray:
    return np.ascontiguousarray(
        Wv.T.reshape(cfg.EC, P, cfg.H).transpose(1, 0, 2).reshape(P, cfg.EC * cfg.H)
    ).astype(np.float16)


def _masks(cfg: Cfg, h: int) -> np.ndarray:
    # [P, 2P]: cols [:P] own-half diagonal tiles (tril), cols [P:] peer-half
    # diagonal tiles (h=0: peer key block 2j+1 > query block 2j -> empty;
    # h=1: peer key block 2j < query block 2j+1 -> full)
    kt = np.arange(P)[:, None]
    qt = np.arange(P)[None, :]
    tril = (kt <= qt).astype(np.float16)
    oth = (np.zeros if h == 0 else np.ones)((P, P), np.float16)
    return np.concatenate([tril, oth], axis=1)


def shard_inputs(cfg: Cfg, x, pos_emb, Wq, Wk, Wv):
    import ml_dtypes

    f8 = ml_dtypes.float8_e4m3
    x = np.asarray(x, dtype=np.float32)
    pos_emb = np.asarray(pos_emb, dtype=np.float32)
    w8 = _w8_layout(cfg, np.asarray(Wk, np.float32), np.asarray(Wq, np.float32), f8)
    wv = _wv_layout(cfg, np.asarray(Wv, np.float32))
    masks = [_masks(cfg, h) for h in range(2)]
    in_maps = []
    for core in range(8):
        b, h = core // 2, core % 2
        rows = _core_rows(cfg, h)
        xr = x[b][rows]
        in_maps.append(
            {
                "x8": _pmajor(cfg, xr, f8),
                "xP": _pmajor(cfg, xr, np.float16),
                "pe8": _pmajor(cfg, pos_emb[b][rows], f8),
                "w8": w8,
                "wv": wv,
                "qmask": masks[h],
            }
        )
    return in_maps


def unshard(cfg: Cfg, results) -> np.ndarray:
    out = np.empty((cfg.B, cfg.T, cfg.H), np.float32)
    for core in range(8):
        b, h = core // 2, core % 2
        rows = _core_rows(cfg, h)
        out[b][rows] = results[core]["outT"].T.astype(np.float32)
    return out


_NC_CACHE = {}


def _get_nc(cfg: Cfg) -> bass.Bass:
    if cfg not in _NC_CACHE:
        nc = build(cfg)
        _dedupe_ldweights(nc)
        _split_sync_waits(nc)
        _NC_CACHE[cfg] = nc
    return _NC_CACHE[cfg]


def kernel(x, pos_emb, Wq, Wk, Wv) -> np.ndarray:
    from concourse.bass_utils import run_bass_kernel_spmd

    cfg = FULL
    nc = _get_nc(cfg)
    in_maps = shard_inputs(cfg, x, pos_emb, Wq, Wk, Wv)
    res = run_bass_kernel_spmd(nc, in_maps, list(range(8)))
    return unshard(cfg, res.results)


# revision 13
# speedup vs baseline: 1.1025x; 1.1025x over previous
"""Trainium2 Bass kernel for a single causal attention head.

  q = x @ Wq.T; k = pos_emb @ Wk.T; v = x @ Wv.T
  out = softmax(causal(q @ k.T / sqrt(E))) @ v

Sharding (8 cores): core c -> (batch b = c//2, half h = c%2). Core h owns the
interleaved 128-row blocks {2j+h} of batch b (queries AND keys) so causal work
is balanced across the pair. Each core projects Q/K/V for its own rows;
attention over the core's OWN key half reads K/V straight from SBUF (no
collective), while the peer half is recovered rank-uniformly as
AllReduce_pair(add) - own on the vector engine. Masks encode the own/other
causal boundary as data, so the program has no rank-dependent control flow.

Precision: all three projections run in fp8(e4m3) DoubleRow mode (two
E-chunks per matmul); scores and attn@v stay fp16 with fp32 PSUM. fp8 V noise
only matters for low-key-count queries (the first 128 rows of each core's
half), so the first local key block's V gets an exact dual-fp8 residual
correction: dv = (xr8*64)@Wv8/64 + x8@((Wv-Wv8)*64)/64, with residuals scaled
by 64 to clear the fp8 subnormal floor. Validated 1.65e-2 max rel err vs fp64
on the reference inputs (identical to an fp16-V implementation).

Softmax denominators come from quad-summed exp tiles (DVE) feeding one
ones-vector matmul per quad. All activations are host-quantized, transposed
and partition-major ([P, EC, T_core]) so every matmul has the contraction dim
on partitions, with multi-KB contiguous DMA descriptors.
"""

import os
import sys
from contextlib import ExitStack
from dataclasses import dataclass

import numpy as np


def _ensure_path():
    try:
        import concourse.bass  # noqa: F401
    except ImportError:
        for p in ("/opt/trn_rl_repo", "/root/.axon_site/_ro/trn_rl_repo"):
            if os.path.isdir(p) and p not in sys.path:
                sys.path.insert(0, p)


_ensure_path()

import concourse.bass as bass  # noqa: E402
import concourse.mybir as mybir  # noqa: E402
import concourse.tile as tile  # noqa: E402
from concourse.masks import make_identity  # noqa: E402

P = 128
F8 = mybir.dt.float8e4
F16 = mybir.dt.float16
F32 = mybir.dt.float32
DR = mybir.MatmulPerfMode.DoubleRow
RSCALE = 64.0  # residual pre-scale (clears fp8 subnormals)


@dataclass(frozen=True)
class Cfg:
    B: int = 4
    T: int = 2048
    E: int = 4096
    H: int = 128
    QGB: int = 4  # 128-blocks per query group (matmul free dim = QGB*P)

    @property
    def NB(self):  # key/query 128-blocks per core per half
        return self.T // (2 * P)

    @property
    def TB(self):  # rows per core
        return self.NB * P

    @property
    def NQG(self):  # query groups per core
        return self.NB // self.QGB

    @property
    def QG(self):  # queries per group
        return self.QGB * P

    @property
    def EC(self):  # contraction chunks
        return self.E // P

    @property
    def NPAIR(self):  # fp8 DoubleRow chunk pairs
        return self.EC // 2


FULL = Cfg()

MAX_SYNC_WAITS = 1


def _dedupe_ldweights(nc: bass.Bass):
    """Drop PE Ldweights whose stationary operand is identical to the weights
    already loaded. Any sync conditions move onto the following PE
    instruction."""
    import orjson

    n = 0
    for fn in nc.m.functions:
        for bb in fn.blocks:
            out = []
            last_sig = None
            pending_sync = None
            for inst in bb.instructions:
                if getattr(inst, "engine", None) != mybir.EngineType.PE:
                    out.append(inst)
                    continue
                d = orjson.loads(nc.instruction_to_json(inst))
                if d["opcode"] == "Ldweights":
                    sig = orjson.dumps(
                        [d.get("ins"), d.get("tile_position"), d.get("tile_size")]
                    )
                    if sig == last_sig:
                        si = inst.sync_info
                        if si and (si.on_wait or si.on_update):
                            pending_sync = si
                        n += 1
                        continue  # drop
                    last_sig = sig
                if pending_sync is not None:
                    si = inst.sync_info
                    if si is None:
                        inst.sync_info = pending_sync
                    else:
                        si.on_wait = list(pending_sync.on_wait) + list(si.on_wait)
                        si.on_update = list(pending_sync.on_update) + list(
                            si.on_update
                        )
                    pending_sync = None
                out.append(inst)
            assert pending_sync is None
            bb.instructions[:] = out
    return n


def _split_sync_waits(nc: bass.Bass, maxw: int = MAX_SYNC_WAITS):
    n = 0
    for fn in nc.m.functions:
        for bb in fn.blocks:
            out = []
            for inst in bb.instructions:
                si = inst.sync_info
                waits = list(si.on_wait) if si and si.on_wait else []
                if len(waits) > maxw:
                    excess, keep = waits[:-maxw], waits[-maxw:]
                    for k in range(0, len(excess), maxw):
                        carrier = mybir.InstNoOp(
                            name=f"{inst.name}-wsplit{n}",
                            engine=inst.engine,
                            ins=[],
                            outs=[],
                            sync_info=mybir.SyncInfo(
                                on_wait=excess[k : k + maxw], on_update=[]
                            ),
                        )
                        n += 1
                        out.append(carrier)
                    si.on_wait = keep
                out.append(inst)
            bb.instructions[:] = out
    return n


def build(cfg: Cfg, mock_cc: bool = False) -> bass.Bass:
    assert cfg.H == P
    TB, NB, EC, QG, QGB, NQG, H, NPAIR = (
        cfg.TB, cfg.NB, cfg.EC, cfg.QG, cfg.QGB, cfg.NQG, cfg.H, cfg.NPAIR,
    )
    KV = TB * H

    nc = bass.Bass("TRN2", target_bir_lowering=False, debug=False, num_devices=8)

    # p-major streams: [p, c, t] so one DMA row per partition covers a whole
    # super-chunk contiguously (multi-KB descriptors)
    pe8 = nc.dram_tensor("pe8", [P, EC, TB], F8, kind="ExternalInput").ap()
    x8 = nc.dram_tensor("x8", [P, EC, TB], F8, kind="ExternalInput").ap()
    # first-local-block x residual, pre-scaled by RSCALE
    xr8 = nc.dram_tensor("xr8", [P, EC, P], F8, kind="ExternalInput").ap()
    # per-pair interleave [wk8 | wq8 | wv8] with [2(j), H] inner layout
    w8 = nc.dram_tensor(
        "w8", [P, NPAIR * 3 * 2 * H], F8, kind="ExternalInput"
    ).ap()
    # V-weight residual, pre-scaled by RSCALE, pair layout [2(j), H]
    wvr8 = nc.dram_tensor(
        "wvr8", [P, NPAIR * 2 * H], F8, kind="ExternalInput"
    ).ap()
    qmask = nc.dram_tensor("qmask", [P, 2 * P], F16, kind="ExternalInput").ap()
    outT = nc.dram_tensor("outT", [H, TB], F16, kind="ExternalOutput").ap()

    cc_k_in = nc.dram_tensor("cc_k_in", [KV], F16).ap()
    cc_k_out = nc.dram_tensor("cc_k_out", [KV], F16).ap()
    cc_v_in = nc.dram_tensor("cc_v_in", [KV], F16).ap()
    cc_v_out = nc.dram_tensor("cc_v_out", [KV], F16).ap()

    scale = 1.0 / np.sqrt(float(cfg.E))
    groups = [[0, 1], [2, 3], [4, 5], [6, 7]]

    # super-chunks in PAIRS of 128-chunks (DoubleRow consumes 2 at a time)
    PSCS = []
    rem = NPAIR
    for want in [1, 1] + [2] * NPAIR:
        if rem == 0:
            break
        s = min(want, rem)
        PSCS.append(s)
        rem -= s
    NSC = len(PSCS)
    POFF = [sum(PSCS[:i]) for i in range(NSC)]

    NT = 2
    FD = 512

    with tile.TileContext(nc) as tc, ExitStack() as ctx:
        consts = ctx.enter_context(tc.tile_pool(name="consts", bufs=1))
        big = ctx.enter_context(tc.tile_pool(name="big", bufs=1))
        pe8_pool = ctx.enter_context(tc.tile_pool(name="pe8p", bufs=4))
        x8_pool = ctx.enter_context(tc.tile_pool(name="x8p", bufs=4))
        e_pool = ctx.enter_context(tc.tile_pool(name="eT", bufs=4 * QGB * NQG + 10))
        sm = ctx.enter_context(tc.tile_pool(name="sm", bufs=2))

        proj_ctx = ExitStack()
        pp = proj_ctx.enter_context(tc.tile_pool(name="pp", bufs=6, space="PSUM"))
        dv_pool = proj_ctx.enter_context(
            tc.tile_pool(name="dvp", bufs=1, space="PSUM")
        )
        tr_ps_pool = proj_ctx.enter_context(
            tc.tile_pool(name="trp", bufs=1, space="PSUM")
        )

        # ---- constants ----
        ones_col = consts.tile([P, 1], F16, tag="ones_col")
        nc.any.memset(ones_col[:], 1.0)
        ones_row = consts.tile([1, P], F32, tag="ones_row")
        nc.any.memset(ones_row[:], 1.0)
        ident = consts.tile([P, P], F16, tag="ident")
        make_identity(nc, ident[:])
        warm = consts.tile([P, 1], F16, tag="warm")
        nc.scalar.activation(
            warm[:], ones_col[:], mybir.ActivationFunctionType.Exp
        )

        w8_sb = consts.tile([P, NPAIR, 3, 2, H], F8, tag="w8")
        wvr_sb = consts.tile([P, NPAIR, 2, H], F8, tag="wvr")
        xr_sb = consts.tile([P, EC, P], F8, tag="xr")
        qm_sb = consts.tile([P, 2 * P], F16, tag="qm")

        # ---- belt DMA queue plan ----
        # scalar (ACT) queue: qmask, xr8, pe8 stream.
        # sync (SP) queue: w8 slices ahead of pace, wvr8, x8 stream, outT tail.
        # gpsimd (SWDGE) queue: collective bounces + readbacks only.
        w8v = w8.rearrange("p (m s j h) -> p m s j h", s=3, j=2, h=H)
        wvrv = wvr8.rearrange("p (m j h) -> p m j h", j=2, h=H)

        nc.scalar.dma_start(qm_sb[:], qmask)
        nc.scalar.dma_start(xr_sb[:], xr8)
        pe8_tiles = {}
        x8_tiles = {}
        for sc in range(NSC):
            n = PSCS[sc]
            off = POFF[sc]
            t = pe8_pool.tile([P, 2 * n, TB], F8, tag="pe8", name=f"pe8_{sc}")
            nc.scalar.dma_start(t[:], pe8[:, 2 * off : 2 * (off + n), :])
            pe8_tiles[sc] = t

        def w8_slice(sc):
            lo, hi = POFF[sc], POFF[sc] + PSCS[sc]
            nc.sync.dma_start(w8_sb[:, lo:hi], w8v[:, lo:hi])

        w8_slice(0)
        w8_slice(1)
        for sc in range(2, NSC):
            w8_slice(sc)
        nc.sync.dma_start(wvr_sb[:], wvrv)
        for sc in range(NSC):
            n = PSCS[sc]
            off = POFF[sc]
            t = x8_pool.tile([P, 2 * n, TB], F8, tag="x8", name=f"x8_{sc}")
            nc.sync.dma_start(t[:], x8[:, 2 * off : 2 * (off + n), :])
            x8_tiles[sc] = t

        k_ps = [pp.tile([P, FD], F32, tag="pp", name=f"k_ps{i}") for i in range(NT)]
        v_ps = [pp.tile([P, FD], F32, tag="pp", name=f"v_ps{i}") for i in range(NT)]
        q_ps = [pp.tile([P, FD], F32, tag="pp", name=f"q_ps{i}") for i in range(NT)]
        d_ps8 = dv_pool.tile([P, P], F32, tag="dv", name="dv_ps")

        def proj8(ps, s, t, sc):
            for i in range(PSCS[sc]):
                m = POFF[sc] + i
                for it in range(NT):
                    nc.tensor.matmul(
                        ps[it][:],
                        w8_sb[:, m, s],
                        t[:, 2 * i : 2 * i + 2, it * FD : (it + 1) * FD],
                        start=(m == 0),
                        stop=(m == NPAIR - 1),
                        perf_mode=DR,
                    )

        def projv_d1(t, sc):
            # V projection with the xr-residual pass sharing each stationary
            for i in range(PSCS[sc]):
                m = POFF[sc] + i
                for it in range(NT):
                    nc.tensor.matmul(
                        v_ps[it][:],
                        w8_sb[:, m, 2],
                        t[:, 2 * i : 2 * i + 2, it * FD : (it + 1) * FD],
                        start=(m == 0),
                        stop=(m == NPAIR - 1),
                        perf_mode=DR,
                    )
                nc.tensor.matmul(
                    d_ps8[:],
                    w8_sb[:, m, 2],
                    xr_sb[:, 2 * i + 2 * POFF[sc] : 2 * i + 2 * POFF[sc] + 2, :],
                    start=(m == 0),
                    stop=False,
                    perf_mode=DR,
                )

        def projd2(t, sc):
            # W-residual pass: dv += x8_blk0 @ wvr8
            for i in range(PSCS[sc]):
                m = POFF[sc] + i
                nc.tensor.matmul(
                    d_ps8[:],
                    wvr_sb[:, m],
                    t[:, 2 * i : 2 * i + 2, 0:P],
                    start=False,
                    stop=(m == NPAIR - 1),
                    perf_mode=DR,
                )

        # ---- phase 1: projections, K leading at pe8-stream pace
        for sc in range(NSC):
            proj8(k_ps, 0, pe8_tiles[sc], sc)
            projv_d1(x8_tiles[sc], sc)
            proj8(q_ps, 1, x8_tiles[sc], sc)
            projd2(x8_tiles[sc], sc)
            if sc == NSC - 1:
                # K done: own half to SBUF, kick the exchange
                with tc.high_priority():
                    kT_own = big.tile([P, TB], F16, tag="kT_own")
                    for i in range(NT):
                        nc.vector.tensor_copy(
                            kT_own[:, i * FD : (i + 1) * FD], k_ps[i][:]
                        )
                    nc.gpsimd.dma_start(
                        cc_k_in.rearrange("(h t) -> h t", t=TB), kT_own[:]
                    )
                    if mock_cc:
                        nc.gpsimd.dma_start(cc_k_out[:], cc_k_in[:])
                    else:
                        nc.gpsimd.collective_compute(
                            "AllReduce",
                            mybir.AluOpType.add,
                            replica_groups=groups,
                            ins=[cc_k_in[:]],
                            outs=[cc_k_out[:]],
                        )

        qT_sb = big.tile([P, TB], F16, tag="qT")
        for i in range(NT):
            nc.vector.tensor_copy(qT_sb[:, i * FD : (i + 1) * FD], q_ps[i][:])

        # ---- V done: transpose to natural layout, exchange, then patch the
        # first local key block with the residual correction (the exchange
        # ships the pure-fp8 copy: every peer query attending these keys has
        # k>=128 so fp8 noise averages out there)
        vT_loc = big.tile([P, TB], F16, tag="vT_loc")
        for i in range(NT):
            nc.vector.tensor_copy(vT_loc[:, i * FD : (i + 1) * FD], v_ps[i][:])
        v_own = big.tile([P, NB, H], F16, tag="v_own")
        for c in range(NB):
            t_ps = tr_ps_pool.tile([P, P], F16, tag="tr")
            nc.tensor.transpose(t_ps[:], vT_loc[:, c * P : (c + 1) * P], ident[:])
            nc.vector.tensor_copy(v_own[:, c, :], t_ps[:])
        nc.gpsimd.dma_start(
            cc_v_in.rearrange("(p c h) -> p c h", p=P, h=H), v_own[:]
        )
        if mock_cc:
            nc.gpsimd.dma_start(cc_v_out[:], cc_v_in[:])
        else:
            nc.gpsimd.collective_compute(
                "AllReduce",
                mybir.AluOpType.add,
                replica_groups=groups,
                ins=[cc_v_in[:]],
                outs=[cc_v_out[:]],
            )

        # dv finalize: unscale, transpose, add into v_own block 0
        dvT_sb = sm.tile([H, P], F16, tag="dvT")
        nc.vector.tensor_scalar_mul(dvT_sb[:], d_ps8[:], 1.0 / RSCALE)
        dv_tr = tr_ps_pool.tile([P, P], F16, tag="tr", name="dv_tr")
        nc.tensor.transpose(dv_tr[:], dvT_sb[:], ident[:])
        nc.vector.tensor_add(v_own[:, 0, :], v_own[:, 0, :], dv_tr[:])

        # readbacks AFTER both CC triggers so a pending CC-K wait never
        # delays the V exchange trigger on the gpsimd sequencer
        with tc.high_priority():
            kT_sum = big.tile([P, TB], F16, tag="kT_sum")
            nc.gpsimd.dma_start(
                kT_sum[:], cc_k_out.rearrange("(h t) -> h t", t=TB)
            )
            kT_oth = big.tile([P, TB], F16, tag="kT_oth")
            nc.vector.tensor_sub(kT_oth[:], kT_sum[:], kT_own[:])
        v_sum = big.tile([P, NB, H], F16, tag="v_sum")
        nc.gpsimd.dma_start(
            v_sum[:], cc_v_out.rearrange("(p c h) -> p c h", p=P, h=H)
        )
        v_oth = big.tile([P, NB, H], F16, tag="v_oth")
        nc.vector.tensor_sub(v_oth[:], v_sum[:], v_own[:])

        # ---- phase 2: attention ----
        # halves: r=0 own keys (SBUF-resident), r=1 peer keys (exchanged).
        # Key order within softmax is irrelevant; qmask cols [:P] mask the
        # own-half diagonal tiles (tril), cols [P:] the peer-half diagonal
        # (zeros for h=0, ones for h=1).
        proj_ctx.close()
        sT_pool = ctx.enter_context(tc.tile_pool(name="sTp", bufs=4, space="PSUM"))
        o_pool = ctx.enter_context(tc.tile_pool(name="op", bufs=2, space="PSUM"))
        d_pool = ctx.enter_context(tc.tile_pool(name="dp", bufs=2, space="PSUM"))

        o_ps = {}
        d_ps = {}
        e_tiles = {g: [] for g in range(NQG)}
        esums = {g: [] for g in range(NQG)}
        nk = {g: QGB * (g + 1) for g in range(NQG)}
        for g in range(NQG):
            o_ps[g] = o_pool.tile([P, QG], F32, tag="o", name=f"o_ps{g}")
            d_ps[g] = d_pool.tile([1, QG], F32, tag="d", name=f"d_ps{g}")

        def mm1_half(r, kT_half):
            # kslot-outer interleave so consecutive MM1s share one Ldweights
            for c in range(NB):
                for g in range(NQG):
                    if c >= nk[g]:
                        continue
                    col0 = (c - QGB * g) * P if c >= QGB * g else 0
                    sT = sT_pool.tile(
                        [P, QG], F32, tag="sT", name=f"sT_{g}_{r}_{c}"
                    )
                    nc.tensor.matmul(
                        sT[:, col0:],
                        kT_half[:, c * P : (c + 1) * P],
                        qT_sb[:, g * QG + col0 : (g + 1) * QG],
                        start=True,
                        stop=True,
                    )
                    eT = e_pool.tile(
                        [P, QG], F16, tag="eT", name=f"eT_{g}_{r}_{c}"
                    )
                    if c >= QGB * g:
                        nc.scalar.activation(
                            eT[:, col0:], sT[:, col0:],
                            mybir.ActivationFunctionType.Exp, scale=scale,
                        )
                        nc.vector.tensor_mul(
                            eT[:, col0 : col0 + P],
                            eT[:, col0 : col0 + P],
                            qm_sb[:, r * P : (r + 1) * P],
                        )
                    else:
                        nc.scalar.activation(
                            eT[:], sT[:], mybir.ActivationFunctionType.Exp,
                            scale=scale,
                        )
                    e_tiles[g].append((r * NB + c, eT, col0))
                    # quad-summed denominators on DVE; col0 is nondecreasing
                    # inside each quad so the partial below col0 stays exact
                    qidx = len(e_tiles[g]) - 1
                    if qidx % QGB == 0:
                        es = e_pool.tile(
                            [P, QG], F16, tag="eT", name=f"es_{g}_{r}_{c}"
                        )
                        nc.vector.tensor_copy(es[:, col0:], eT[:, col0:])
                        esums[g].append((es, col0))
                    else:
                        es, _ = esums[g][-1]
                        nc.vector.tensor_add(
                            es[:, col0:], es[:, col0:], eT[:, col0:]
                        )

        nq_half = {g: -(-nk[g] // QGB) for g in range(NQG)}
        mm2_idx = {g: 0 for g in range(NQG)}

        def mm2_flush(g, new):
            for es, col0 in new:
                nc.tensor.matmul(
                    d_ps[g][:], ones_col[:], es[:],
                    start=(mm2_idx[g] == 0),
                    stop=(mm2_idx[g] == 2 * nq_half[g] - 1),
                )
                mm2_idx[g] += 1

        mm3_idx = {g: 0 for g in range(NQG)}
        n_mm3 = {g: 2 * nk[g] for g in range(NQG)}

        def mm3_half(r, v_half):
            for c in range(NB):
                for g in range(NQG):
                    if c >= nk[g]:
                        continue
                    kslot = r * NB + c
                    eT, col0 = next(
                        (e, c0) for (ks, e, c0) in e_tiles[g] if ks == kslot
                    )
                    nc.tensor.matmul(
                        o_ps[g][:, col0:], v_half[:, c, :], eT[:, col0:],
                        start=(mm3_idx[g] == 0),
                        stop=(mm3_idx[g] == n_mm3[g] - 1),
                    )
                    mm3_idx[g] += 1

        # PE order: own scores -> own denominators -> own attn@v (local) ->
        # peer scores (waits the K exchange) -> peer denominators -> peer
        # attn@v (waits the V exchange)
        mm1_half(0, kT_own)
        for g in range(NQG):
            mm2_flush(g, esums[g])
        mm3_half(0, v_own)
        mm1_half(1, kT_oth)
        for g in range(NQG):
            mm2_flush(g, esums[g][nq_half[g] :])
        mm3_half(1, v_oth)

        for g in range(NQG):
            rec = sm.tile([1, QG], F32, tag="rec", name=f"rec{g}")
            nc.vector.reciprocal(rec[:], d_ps[g][:])
            bc_ps = sT_pool.tile([P, QG], F32, tag="sT", name=f"bc_ps{g}")
            nc.tensor.matmul(bc_ps[:], ones_row[:], rec[:], start=True, stop=True)
            bc_sb = sm.tile([P, QG], F32, tag="bcs", name=f"bc_sb{g}")
            nc.vector.tensor_copy(bc_sb[:], bc_ps[:])
            oT = sm.tile([P, QG], F16, tag="oT", name=f"oT{g}")
            nc.vector.tensor_mul(oT[:], o_ps[g][:], bc_sb[:])
            nc.sync.dma_start(outT[:, g * QG : (g + 1) * QG], oT[:])

    return nc


def _core_rows(cfg: Cfg, h: int) -> np.ndarray:
    j = np.arange(cfg.TB)
    return ((j // P) * 2 + h) * P + (j % P)


def _pmajor(cfg: Cfg, a: np.ndarray, dt) -> np.ndarray:
    # [rows, E] -> [P, EC, rows] with [p, c, t] = a[t, c*P + p]
    nrow = a.shape[0]
    return np.ascontiguousarray(
        a.T.reshape(cfg.EC, P, nrow).transpose(1, 0, 2)
    ).astype(dt)


def _w8_layout(cfg: Cfg, Wk, Wq, Wv8, dt) -> np.ndarray:
    # [P, NPAIR, 3(k,q,v), 2(j), H] with [p, m, s, j, h] = W_s[h, (2m+j)P+p]
    def lay(W):  # [P, NPAIR, 2, H]
        return W.T.reshape(cfg.NPAIR, 2, P, cfg.H).transpose(2, 0, 1, 3)

    out = np.empty((P, cfg.NPAIR, 3, 2, cfg.H), np.float32)
    out[:, :, 0] = lay(Wk)
    out[:, :, 1] = lay(Wq)
    out[:, :, 2] = lay(Wv8)
    return np.ascontiguousarray(
        out.reshape(P, cfg.NPAIR * 3 * 2 * cfg.H)
    ).astype(dt)


def _masks(cfg: Cfg, h: int) -> np.ndarray:
    # [P, 2P]: cols [:P] own-half diagonal tiles (tril), cols [P:] peer-half
    # diagonal tiles (h=0: empty; h=1: full)
    kt = np.arange(P)[:, None]
    qt = np.arange(P)[None, :]
    tril = (kt <= qt).astype(np.float16)
    oth = (np.zeros if h == 0 else np.ones)((P, P), np.float16)
    return np.concatenate([tril, oth], axis=1)


def shard_inputs(cfg: Cfg, x, pos_emb, Wq, Wk, Wv):
    import ml_dtypes

    f8 = ml_dtypes.float8_e4m3
    x = np.asarray(x, dtype=np.float32)
    pos_emb = np.asarray(pos_emb, dtype=np.float32)
    Wv = np.asarray(Wv, np.float32)
    Wv8 = Wv.astype(f8)
    wvr = ((Wv - Wv8.astype(np.float32)) * RSCALE).astype(f8)
    w8 = _w8_layout(
        cfg, np.asarray(Wk, np.float32), np.asarray(Wq, np.float32),
        Wv8.astype(np.float32), f8,
    )
    # wvr8 pair layout [P, NPAIR, 2, H]
    wvr8 = np.ascontiguousarray(
        wvr.astype(np.float32).T.reshape(cfg.NPAIR, 2, P, cfg.H)
        .transpose(2, 0, 1, 3).reshape(P, cfg.NPAIR * 2 * cfg.H)
    ).astype(f8)
    masks = [_masks(cfg, h) for h in range(2)]
    in_maps = []
    for core in range(8):
        b, h = core // 2, core % 2
        rows = _core_rows(cfg, h)
        xr = x[b][rows]
        x8f = xr.astype(f8)
        xblk = xr[:P]
        xres = ((xblk - x8f[:P].astype(np.float32)) * RSCALE).astype(f8)
        in_maps.append(
            {
                "x8": _pmajor(cfg, x8f.astype(np.float32), f8),
                "xr8": _pmajor(cfg, xres.astype(np.float32), f8),
                "pe8": _pmajor(cfg, pos_emb[b][rows], f8),
                "w8": w8,
                "wvr8": wvr8,
                "qmask": masks[h],
            }
        )
    return in_maps


def unshard(cfg: Cfg, results) -> np.ndarray:
    out = np.empty((cfg.B, cfg.T, cfg.H), np.float32)
    for core in range(8):
        b, h = core // 2, core % 2
        rows = _core_rows(cfg, h)
        out[b][rows] = results[core]["outT"].T.astype(np.float32)
    return out


_NC_CACHE = {}


def _get_nc(cfg: Cfg) -> bass.Bass:
    if cfg not in _NC_CACHE:
        nc = build(cfg)
        _dedupe_ldweights(nc)
        _split_sync_waits(nc)
        _NC_CACHE[cfg] = nc
    return _NC_CACHE[cfg]


def kernel(x, pos_emb, Wq, Wk, Wv) -> np.ndarray:
    from concourse.bass_utils import run_bass_kernel_spmd

    cfg = FULL
    nc = _get_nc(cfg)
    in_maps = shard_inputs(cfg, x, pos_emb, Wq, Wk, Wv)
    res = run_bass_kernel_spmd(nc, in_maps, list(range(8)))
    return unshard(cfg, res.results)


# revision 15
# speedup vs baseline: 1.2277x; 1.1135x over previous
"""Trainium2 Bass kernel for a single causal attention head.

  q = x @ Wq.T; k = pos_emb @ Wk.T; v = x @ Wv.T
  out = softmax(causal(q @ k.T / sqrt(E))) @ v

Sharding (8 cores): core c -> (batch b = c//2, half h = c%2). Core h owns the
interleaved 128-row blocks {2j+h} of batch b (queries AND keys) so causal work
is balanced across the pair. Each core projects Q/K/V for its own rows;
attention over the core's OWN key half reads K/V straight from SBUF (no
collective), while the peer half is recovered rank-uniformly as
AllReduce_pair(add) - own on the vector engine. Masks encode the own/other
causal boundary as data, so the program has no rank-dependent control flow.

Precision: all three projections run in fp8(e4m3) DoubleRow mode (two
E-chunks per matmul); scores and attn@v stay fp16 with fp32 PSUM. fp8 V noise
only matters for low-key-count queries (the first 128 rows of each core's
half), so the first local key block's V gets an exact dual-fp8 residual
correction: dv = (xr8*64)@Wv8/64 + x8@((Wv-Wv8)*64)/64, with residuals scaled
by 64 to clear the fp8 subnormal floor. Validated 1.65e-2 max rel err vs fp64
on the reference inputs (identical to an fp16-V implementation).

Softmax denominators come from quad-summed exp tiles (DVE) feeding one
ones-vector matmul per quad. All activations are host-quantized, transposed
and partition-major ([P, EC, T_core]) so every matmul has the contraction dim
on partitions, with multi-KB contiguous DMA descriptors.
"""

import os
import sys
from contextlib import ExitStack
from dataclasses import dataclass

import numpy as np


def _ensure_path():
    try:
        import concourse.bass  # noqa: F401
    except ImportError:
        for p in ("/opt/trn_rl_repo", "/root/.axon_site/_ro/trn_rl_repo"):
            if os.path.isdir(p) and p not in sys.path:
                sys.path.insert(0, p)


_ensure_path()

import concourse.bass as bass  # noqa: E402
import concourse.mybir as mybir  # noqa: E402
import concourse.tile as tile  # noqa: E402
from concourse.masks import make_identity  # noqa: E402

P = 128
F8 = mybir.dt.float8e4
F16 = mybir.dt.float16
F32 = mybir.dt.float32
DR = mybir.MatmulPerfMode.DoubleRow
RSCALE = 64.0  # residual pre-scale (clears fp8 subnormals)


@dataclass(frozen=True)
class Cfg:
    B: int = 4
    T: int = 2048
    E: int = 4096
    H: int = 128
    QGB: int = 4  # 128-blocks per query group (matmul free dim = QGB*P)

    @property
    def NB(self):  # key/query 128-blocks per core per half
        return self.T // (2 * P)

    @property
    def TB(self):  # rows per core
        return self.NB * P

    @property
    def NQG(self):  # query groups per core
        return self.NB // self.QGB

    @property
    def QG(self):  # queries per group
        return self.QGB * P

    @property
    def EC(self):  # contraction chunks
        return self.E // P

    @property
    def NPAIR(self):  # fp8 DoubleRow chunk pairs
        return self.EC // 2


FULL = Cfg()

MAX_SYNC_WAITS = 1


def _dedupe_ldweights(nc: bass.Bass):
    """Drop PE Ldweights whose stationary operand is identical to the weights
    already loaded. Any sync conditions move onto the following PE
    instruction."""
    import orjson

    n = 0
    for fn in nc.m.functions:
        for bb in fn.blocks:
            out = []
            last_sig = None
            pending_sync = None
            for inst in bb.instructions:
                if getattr(inst, "engine", None) != mybir.EngineType.PE:
                    out.append(inst)
                    continue
                d = orjson.loads(nc.instruction_to_json(inst))
                if d["opcode"] == "Ldweights":
                    sig = orjson.dumps(
                        [d.get("ins"), d.get("tile_position"), d.get("tile_size")]
                    )
                    if sig == last_sig:
                        si = inst.sync_info
                        if si and (si.on_wait or si.on_update):
                            pending_sync = si
                        n += 1
                        continue  # drop
                    last_sig = sig
                if pending_sync is not None:
                    si = inst.sync_info
                    if si is None:
                        inst.sync_info = pending_sync
                    else:
                        si.on_wait = list(pending_sync.on_wait) + list(si.on_wait)
                        si.on_update = list(pending_sync.on_update) + list(
                            si.on_update
                        )
                    pending_sync = None
                out.append(inst)
            assert pending_sync is None
            bb.instructions[:] = out
    return n


def _split_sync_waits(nc: bass.Bass, maxw: int = MAX_SYNC_WAITS):
    n = 0
    for fn in nc.m.functions:
        for bb in fn.blocks:
            out = []
            for inst in bb.instructions:
                si = inst.sync_info
                waits = list(si.on_wait) if si and si.on_wait else []
                if len(waits) > maxw:
                    excess, keep = waits[:-maxw], waits[-maxw:]
                    for k in range(0, len(excess), maxw):
                        carrier = mybir.InstNoOp(
                            name=f"{inst.name}-wsplit{n}",
                            engine=inst.engine,
                            ins=[],
                            outs=[],
                            sync_info=mybir.SyncInfo(
                                on_wait=excess[k : k + maxw], on_update=[]
                            ),
                        )
                        n += 1
                        out.append(carrier)
                    si.on_wait = keep
                out.append(inst)
            bb.instructions[:] = out
    return n


def build(cfg: Cfg, mock_cc: bool = False) -> bass.Bass:
    assert cfg.H == P
    TB, NB, EC, QG, QGB, NQG, H, NPAIR = (
        cfg.TB, cfg.NB, cfg.EC, cfg.QG, cfg.QGB, cfg.NQG, cfg.H, cfg.NPAIR,
    )
    KV = TB * H

    nc = bass.Bass("TRN2", target_bir_lowering=False, debug=False, num_devices=8)

    # p-major streams: [p, c, t] so one DMA row per partition covers a whole
    # super-chunk contiguously (multi-KB descriptors)
    pe8 = nc.dram_tensor("pe8", [P, EC, TB], F8, kind="ExternalInput").ap()
    x8 = nc.dram_tensor("x8", [P, EC, TB], F8, kind="ExternalInput").ap()
    # first-local-block x residual, pre-scaled by RSCALE
    xr8 = nc.dram_tensor("xr8", [P, EC, P], F8, kind="ExternalInput").ap()
    # per-pair interleave [wk8 | wq8 | wv8] with [2(j), H] inner layout
    w8 = nc.dram_tensor(
        "w8", [P, NPAIR * 3 * 2 * H], F8, kind="ExternalInput"
    ).ap()
    # V-weight residual, pre-scaled by RSCALE, pair layout [2(j), H]
    wvr8 = nc.dram_tensor(
        "wvr8", [P, NPAIR * 2 * H], F8, kind="ExternalInput"
    ).ap()
    qmask = nc.dram_tensor("qmask", [P, 2 * P], F16, kind="ExternalInput").ap()
    outT = nc.dram_tensor("outT", [H, TB], F16, kind="ExternalOutput").ap()

    cc_k_in = nc.dram_tensor("cc_k_in", [KV], F16).ap()
    cc_k_out = nc.dram_tensor("cc_k_out", [KV], F16).ap()
    cc_v_in = nc.dram_tensor("cc_v_in", [KV], F16).ap()
    cc_v_out = nc.dram_tensor("cc_v_out", [KV], F16).ap()

    scale = 1.0 / np.sqrt(float(cfg.E))
    groups = [[0, 1], [2, 3], [4, 5], [6, 7]]

    # super-chunks in PAIRS of 128-chunks (DoubleRow consumes 2 at a time)
    PSCS = []
    rem = NPAIR
    for want in [1, 1] + [2] * NPAIR:
        if rem == 0:
            break
        s = min(want, rem)
        PSCS.append(s)
        rem -= s
    NSC = len(PSCS)
    POFF = [sum(PSCS[:i]) for i in range(NSC)]

    NT = 2
    FD = 512

    with tile.TileContext(nc) as tc, ExitStack() as ctx:
        consts = ctx.enter_context(tc.tile_pool(name="consts", bufs=1))
        big = ctx.enter_context(tc.tile_pool(name="big", bufs=1))
        pe8_pool = ctx.enter_context(tc.tile_pool(name="pe8p", bufs=4))
        x8_pool = ctx.enter_context(tc.tile_pool(name="x8p", bufs=4))
        e_pool = ctx.enter_context(tc.tile_pool(name="eT", bufs=4 * QGB * NQG + 10))
        sm = ctx.enter_context(tc.tile_pool(name="sm", bufs=2))

        proj_ctx = ExitStack()
        pp = proj_ctx.enter_context(tc.tile_pool(name="pp", bufs=6, space="PSUM"))
        dv_pool = proj_ctx.enter_context(
            tc.tile_pool(name="dvp", bufs=1, space="PSUM")
        )
        tr_ps_pool = proj_ctx.enter_context(
            tc.tile_pool(name="trp", bufs=1, space="PSUM")
        )

        # ---- constants ----
        ones_col = consts.tile([P, 1], F16, tag="ones_col")
        nc.any.memset(ones_col[:], 1.0)
        ones_row = consts.tile([1, P], F32, tag="ones_row")
        nc.any.memset(ones_row[:], 1.0)
        ident = consts.tile([P, P], F16, tag="ident")
        make_identity(nc, ident[:])
        warm = consts.tile([P, 1], F16, tag="warm")
        nc.scalar.activation(
            warm[:], ones_col[:], mybir.ActivationFunctionType.Exp
        )

        w8_sb = consts.tile([P, NPAIR, 3, 2, H], F8, tag="w8")
        wvr_sb = consts.tile([P, NPAIR, 2, H], F8, tag="wvr")
        xr_sb = consts.tile([P, EC, P], F8, tag="xr")
        qm_sb = consts.tile([P, 2 * P], F16, tag="qm")

        # ---- belt DMA queue plan ----
        # scalar (ACT) queue: qmask, xr8, pe8 stream.
        # sync (SP) queue: w8 slices ahead of pace, wvr8, x8 stream, outT tail.
        # gpsimd (SWDGE) queue: collective bounces + readbacks only.
        w8v = w8.rearrange("p (m s j h) -> p m s j h", s=3, j=2, h=H)
        wvrv = wvr8.rearrange("p (m j h) -> p m j h", j=2, h=H)

        # pe8 and x8 are split even/odd super-chunks across the two HWDGE
        # queues so the K stream finishes in ~half the belt (its exchange
        # trigger must land early enough to hide the first-collective warmup
        # under the CC stream's init barrier)
        nc.scalar.dma_start(qm_sb[:], qmask)

        def w8_slice(sc):
            lo, hi = POFF[sc], POFF[sc] + PSCS[sc]
            nc.sync.dma_start(w8_sb[:, lo:hi], w8v[:, lo:hi])

        # even scs ride sync (w8 slices staying ~2 scs ahead), odd scs ride
        # scalar (behind qmask); tiles allocate in consumption order so pool
        # reuse never inverts the stream pacing
        def eng_of(sc):
            return nc.sync if sc % 2 == 0 else nc.scalar

        def stream(dram, pool, tag, head=None):
            tiles = {}
            for sc in range(NSC):
                if head is not None:
                    head(sc)
                n = PSCS[sc]
                off = POFF[sc]
                t = pool.tile([P, 2 * n, TB], F8, tag=tag, name=f"{tag}_{sc}")
                eng_of(sc).dma_start(t[:], dram[:, 2 * off : 2 * (off + n), :])
                tiles[sc] = t
            return tiles

        w8_done = [0]

        def w8_head(sc):
            want = min(sc + 2, NSC)
            while w8_done[0] < want:
                w8_slice(w8_done[0])
                w8_done[0] += 1

        pe8_tiles = stream(pe8, pe8_pool, "pe8", head=w8_head)
        nc.sync.dma_start(wvr_sb[:], wvrv)
        nc.scalar.dma_start(xr_sb[:], xr8)
        x8_tiles = stream(x8, x8_pool, "x8")

        k_ps = [pp.tile([P, FD], F32, tag="pp", name=f"k_ps{i}") for i in range(NT)]
        v_ps = [pp.tile([P, FD], F32, tag="pp", name=f"v_ps{i}") for i in range(NT)]
        q_ps = [pp.tile([P, FD], F32, tag="pp", name=f"q_ps{i}") for i in range(NT)]
        d_ps8 = dv_pool.tile([P, P], F32, tag="dv", name="dv_ps")

        def proj8(ps, s, t, sc):
            for i in range(PSCS[sc]):
                m = POFF[sc] + i
                for it in range(NT):
                    nc.tensor.matmul(
                        ps[it][:],
                        w8_sb[:, m, s],
                        t[:, 2 * i : 2 * i + 2, it * FD : (it + 1) * FD],
                        start=(m == 0),
                        stop=(m == NPAIR - 1),
                        perf_mode=DR,
                    )

        def projv_d1(t, sc):
            # V projection with the xr-residual pass sharing each stationary
            for i in range(PSCS[sc]):
                m = POFF[sc] + i
                for it in range(NT):
                    nc.tensor.matmul(
                        v_ps[it][:],
                        w8_sb[:, m, 2],
                        t[:, 2 * i : 2 * i + 2, it * FD : (it + 1) * FD],
                        start=(m == 0),
                        stop=(m == NPAIR - 1),
                        perf_mode=DR,
                    )
                nc.tensor.matmul(
                    d_ps8[:],
                    w8_sb[:, m, 2],
                    xr_sb[:, 2 * i + 2 * POFF[sc] : 2 * i + 2 * POFF[sc] + 2, :],
                    start=(m == 0),
                    stop=False,
                    perf_mode=DR,
                )

        def projd2(t, sc):
            # W-residual pass: dv += x8_blk0 @ wvr8
            for i in range(PSCS[sc]):
                m = POFF[sc] + i
                nc.tensor.matmul(
                    d_ps8[:],
                    wvr_sb[:, m],
                    t[:, 2 * i : 2 * i + 2, 0:P],
                    start=False,
                    stop=(m == NPAIR - 1),
                    perf_mode=DR,
                )

        # ---- phase 1: projections, K leading at pe8-stream pace
        for sc in range(NSC):
            proj8(k_ps, 0, pe8_tiles[sc], sc)
            projv_d1(x8_tiles[sc], sc)
            proj8(q_ps, 1, x8_tiles[sc], sc)
            projd2(x8_tiles[sc], sc)
            if sc == NSC - 1:
                # K done: own half to SBUF, kick the exchange
                with tc.high_priority():
                    kT_own = big.tile([P, TB], F16, tag="kT_own")
                    for i in range(NT):
                        nc.vector.tensor_copy(
                            kT_own[:, i * FD : (i + 1) * FD], k_ps[i][:]
                        )
                    nc.gpsimd.dma_start(
                        cc_k_in.rearrange("(h t) -> h t", t=TB), kT_own[:]
                    )
                    if mock_cc:
                        nc.gpsimd.dma_start(cc_k_out[:], cc_k_in[:])
                    else:
                        nc.gpsimd.collective_compute(
                            "AllReduce",
                            mybir.AluOpType.add,
                            replica_groups=groups,
                            ins=[cc_k_in[:]],
                            outs=[cc_k_out[:]],
                        )

        qT_sb = big.tile([P, TB], F16, tag="qT")
        for i in range(NT):
            nc.vector.tensor_copy(qT_sb[:, i * FD : (i + 1) * FD], q_ps[i][:])

        # ---- V done: transpose to natural layout, exchange, then patch the
        # first local key block with the residual correction (the exchange
        # ships the pure-fp8 copy: every peer query attending these keys has
        # k>=128 so fp8 noise averages out there)
        vT_loc = big.tile([P, TB], F16, tag="vT_loc")
        for i in range(NT):
            nc.vector.tensor_copy(vT_loc[:, i * FD : (i + 1) * FD], v_ps[i][:])
        v_own = big.tile([P, NB, H], F16, tag="v_own")
        for c in range(NB):
            t_ps = tr_ps_pool.tile([P, P], F16, tag="tr")
            nc.tensor.transpose(t_ps[:], vT_loc[:, c * P : (c + 1) * P], ident[:])
            nc.vector.tensor_copy(v_own[:, c, :], t_ps[:])
        nc.gpsimd.dma_start(
            cc_v_in.rearrange("(p c h) -> p c h", p=P, h=H), v_own[:]
        )
        if mock_cc:
            nc.gpsimd.dma_start(cc_v_out[:], cc_v_in[:])
        else:
            nc.gpsimd.collective_compute(
                "AllReduce",
                mybir.AluOpType.add,
                replica_groups=groups,
                ins=[cc_v_in[:]],
                outs=[cc_v_out[:]],
            )

        # dv finalize: unscale, transpose, add into v_own block 0
        dvT_sb = sm.tile([H, P], F16, tag="dvT")
        nc.vector.tensor_scalar_mul(dvT_sb[:], d_ps8[:], 1.0 / RSCALE)
        dv_tr = tr_ps_pool.tile([P, P], F16, tag="tr", name="dv_tr")
        nc.tensor.transpose(dv_tr[:], dvT_sb[:], ident[:])
        nc.vector.tensor_add(v_own[:, 0, :], v_own[:, 0, :], dv_tr[:])

        # readbacks AFTER both CC triggers so a pending CC-K wait never
        # delays the V exchange trigger on the gpsimd sequencer
        with tc.high_priority():
            kT_sum = big.tile([P, TB], F16, tag="kT_sum")
            nc.gpsimd.dma_start(
                kT_sum[:], cc_k_out.rearrange("(h t) -> h t", t=TB)
            )
            kT_oth = big.tile([P, TB], F16, tag="kT_oth")
            nc.vector.tensor_sub(kT_oth[:], kT_sum[:], kT_own[:])
        v_sum = big.tile([P, NB, H], F16, tag="v_sum")
        nc.gpsimd.dma_start(
            v_sum[:], cc_v_out.rearrange("(p c h) -> p c h", p=P, h=H)
        )
        v_oth = big.tile([P, NB, H], F16, tag="v_oth")
        nc.vector.tensor_sub(v_oth[:], v_sum[:], v_own[:])

        # ---- phase 2: attention ----
        # halves: r=0 own keys (SBUF-resident), r=1 peer keys (exchanged).
        # Key order within softmax is irrelevant; qmask cols [:P] mask the
        # own-half diagonal tiles (tril), cols [P:] the peer-half diagonal
        # (zeros for h=0, ones for h=1).
        proj_ctx.close()
        sT_pool = ctx.enter_context(tc.tile_pool(name="sTp", bufs=4, space="PSUM"))
        o_pool = ctx.enter_context(tc.tile_pool(name="op", bufs=2, space="PSUM"))
        d_pool = ctx.enter_context(tc.tile_pool(name="dp", bufs=2, space="PSUM"))

        o_ps = {}
        d_ps = {}
        e_tiles = {g: [] for g in range(NQG)}
        esums = {g: [] for g in range(NQG)}
        nk = {g: QGB * (g + 1) for g in range(NQG)}
        for g in range(NQG):
            o_ps[g] = o_pool.tile([P, QG], F32, tag="o", name=f"o_ps{g}")
            d_ps[g] = d_pool.tile([1, QG], F32, tag="d", name=f"d_ps{g}")

        def mm1_half(r, kT_half):
            # kslot-outer interleave so consecutive MM1s share one Ldweights
            for c in range(NB):
                for g in range(NQG):
                    if c >= nk[g]:
                        continue
                    col0 = (c - QGB * g) * P if c >= QGB * g else 0
                    sT = sT_pool.tile(
                        [P, QG], F32, tag="sT", name=f"sT_{g}_{r}_{c}"
                    )
                    nc.tensor.matmul(
                        sT[:, col0:],
                        kT_half[:, c * P : (c + 1) * P],
                        qT_sb[:, g * QG + col0 : (g + 1) * QG],
                        start=True,
                        stop=True,
                    )
                    eT = e_pool.tile(
                        [P, QG], F16, tag="eT", name=f"eT_{g}_{r}_{c}"
                    )
                    if c >= QGB * g:
                        nc.scalar.activation(
                            eT[:, col0:], sT[:, col0:],
                            mybir.ActivationFunctionType.Exp, scale=scale,
                        )
                        nc.vector.tensor_mul(
                            eT[:, col0 : col0 + P],
                            eT[:, col0 : col0 + P],
                            qm_sb[:, r * P : (r + 1) * P],
                        )
                    else:
                        nc.scalar.activation(
                            eT[:], sT[:], mybir.ActivationFunctionType.Exp,
                            scale=scale,
                        )
                    e_tiles[g].append((r * NB + c, eT, col0))
                    # quad-summed denominators on DVE; col0 is nondecreasing
                    # inside each quad so the partial below col0 stays exact
                    qidx = len(e_tiles[g]) - 1
                    if qidx % QGB == 0:
                        es = e_pool.tile(
                            [P, QG], F16, tag="eT", name=f"es_{g}_{r}_{c}"
                        )
                        nc.vector.tensor_copy(es[:, col0:], eT[:, col0:])
                        esums[g].append((es, col0))
                    else:
                        es, _ = esums[g][-1]
                        nc.vector.tensor_add(
                            es[:, col0:], es[:, col0:], eT[:, col0:]
                        )

        nq_half = {g: -(-nk[g] // QGB) for g in range(NQG)}
        mm2_idx = {g: 0 for g in range(NQG)}

        def mm2_flush(g, new):
            for es, col0 in new:
                nc.tensor.matmul(
                    d_ps[g][:], ones_col[:], es[:],
                    start=(mm2_idx[g] == 0),
                    stop=(mm2_idx[g] == 2 * nq_half[g] - 1),
                )
                mm2_idx[g] += 1

        mm3_idx = {g: 0 for g in range(NQG)}
        n_mm3 = {g: 2 * nk[g] for g in range(NQG)}

        def mm3_half(r, v_half):
            for c in range(NB):
                for g in range(NQG):
                    if c >= nk[g]:
                        continue
                    kslot = r * NB + c
                    eT, col0 = next(
                        (e, c0) for (ks, e, c0) in e_tiles[g] if ks == kslot
                    )
                    nc.tensor.matmul(
                        o_ps[g][:, col0:], v_half[:, c, :], eT[:, col0:],
                        start=(mm3_idx[g] == 0),
                        stop=(mm3_idx[g] == n_mm3[g] - 1),
                    )
                    mm3_idx[g] += 1

        # PE order: own scores -> own denominators -> own attn@v (local) ->
        # peer scores (waits the K exchange) -> peer denominators -> peer
        # attn@v (waits the V exchange)
        mm1_half(0, kT_own)
        for g in range(NQG):
            mm2_flush(g, esums[g])
        mm3_half(0, v_own)
        mm1_half(1, kT_oth)
        for g in range(NQG):
            mm2_flush(g, esums[g][nq_half[g] :])
        mm3_half(1, v_oth)

        for g in range(NQG):
            rec = sm.tile([1, QG], F32, tag="rec", name=f"rec{g}")
            nc.vector.reciprocal(rec[:], d_ps[g][:])
            bc_ps = sT_pool.tile([P, QG], F32, tag="sT", name=f"bc_ps{g}")
            nc.tensor.matmul(bc_ps[:], ones_row[:], rec[:], start=True, stop=True)
            bc_sb = sm.tile([P, QG], F32, tag="bcs", name=f"bc_sb{g}")
            nc.vector.tensor_copy(bc_sb[:], bc_ps[:])
            oT = sm.tile([P, QG], F16, tag="oT", name=f"oT{g}")
            nc.vector.tensor_mul(oT[:], o_ps[g][:], bc_sb[:])
            nc.sync.dma_start(outT[:, g * QG : (g + 1) * QG], oT[:])

    return nc


def _core_rows(cfg: Cfg, h: int) -> np.ndarray:
    j = np.arange(cfg.TB)
    return ((j // P) * 2 + h) * P + (j % P)


def _pmajor(cfg: Cfg, a: np.ndarray, dt) -> np.ndarray:
    # [rows, E] -> [P, EC, rows] with [p, c, t] = a[t, c*P + p]
    nrow = a.shape[0]
    return np.ascontiguousarray(
        a.T.reshape(cfg.EC, P, nrow).transpose(1, 0, 2)
    ).astype(dt)


def _w8_layout(cfg: Cfg, Wk, Wq, Wv8, dt) -> np.ndarray:
    # [P, NPAIR, 3(k,q,v), 2(j), H] with [p, m, s, j, h] = W_s[h, (2m+j)P+p]
    def lay(W):  # [P, NPAIR, 2, H]
        return W.T.reshape(cfg.NPAIR, 2, P, cfg.H).transpose(2, 0, 1, 3)

    out = np.empty((P, cfg.NPAIR, 3, 2, cfg.H), np.float32)
    out[:, :, 0] = lay(Wk)
    out[:, :, 1] = lay(Wq)
    out[:, :, 2] = lay(Wv8)
    return np.ascontiguousarray(
        out.reshape(P, cfg.NPAIR * 3 * 2 * cfg.H)
    ).astype(dt)


def _masks(cfg: Cfg, h: int) -> np.ndarray:
    # [P, 2P]: cols [:P] own-half diagonal tiles (tril), cols [P:] peer-half
    # diagonal tiles (h=0: empty; h=1: full)
    kt = np.arange(P)[:, None]
    qt = np.arange(P)[None, :]
    tril = (kt <= qt).astype(np.float16)
    oth = (np.zeros if h == 0 else np.ones)((P, P), np.float16)
    return np.concatenate([tril, oth], axis=1)


def shard_inputs(cfg: Cfg, x, pos_emb, Wq, Wk, Wv):
    import ml_dtypes

    f8 = ml_dtypes.float8_e4m3
    x = np.asarray(x, dtype=np.float32)
    pos_emb = np.asarray(pos_emb, dtype=np.float32)
    Wv = np.asarray(Wv, np.float32)
    Wv8 = Wv.astype(f8)
    wvr = ((Wv - Wv8.astype(np.float32)) * RSCALE).astype(f8)
    w8 = _w8_layout(
        cfg, np.asarray(Wk, np.float32), np.asarray(Wq, np.float32),
        Wv8.astype(np.float32), f8,
    )
    # wvr8 pair layout [P, NPAIR, 2, H]
    wvr8 = np.ascontiguousarray(
        wvr.astype(np.float32).T.reshape(cfg.NPAIR, 2, P, cfg.H)
        .transpose(2, 0, 1, 3).reshape(P, cfg.NPAIR * 2 * cfg.H)
    ).astype(f8)
    masks = [_masks(cfg, h) for h in range(2)]
    in_maps = []
    for core in range(8):
        b, h = core // 2, core % 2
        rows = _core_rows(cfg, h)
        xr = x[b][rows]
        x8f = xr.astype(f8)
        xblk = xr[:P]
        xres = ((xblk - x8f[:P].astype(np.float32)) * RSCALE).astype(f8)
        in_maps.append(
            {
                "x8": _pmajor(cfg, x8f.astype(np.float32), f8),
                "xr8": _pmajor(cfg, xres.astype(np.float32), f8),
                "pe8": _pmajor(cfg, pos_emb[b][rows], f8),
                "w8": w8,
                "wvr8": wvr8,
                "qmask": masks[h],
            }
        )
    return in_maps


def unshard(cfg: Cfg, results) -> np.ndarray:
    out = np.empty((cfg.B, cfg.T, cfg.H), np.float32)
    for core in range(8):
        b, h = core // 2, core % 2
        rows = _core_rows(cfg, h)
        out[b][rows] = results[core]["outT"].T.astype(np.float32)
    return out


_NC_CACHE = {}


def _get_nc(cfg: Cfg) -> bass.Bass:
    if cfg not in _NC_CACHE:
        nc = build(cfg)
        _dedupe_ldweights(nc)
        _split_sync_waits(nc)
        _NC_CACHE[cfg] = nc
    return _NC_CACHE[cfg]


def kernel(x, pos_emb, Wq, Wk, Wv) -> np.ndarray:
    from concourse.bass_utils import run_bass_kernel_spmd

    cfg = FULL
    nc = _get_nc(cfg)
    in_maps = shard_inputs(cfg, x, pos_emb, Wq, Wk, Wv)
    res = run_bass_kernel_spmd(nc, in_maps, list(range(8)))
    return unshard(cfg, res.results)
